# revision 54
# baseline (speedup 1.0000x reference)
"""Dense MoE (all-experts, gate-weighted sum) on 8 Trainium2 NeuronCores.

Sharding: pure data-parallel over the token axis N (8192 -> 1024 rows/core);
every core holds all 8 experts, so no collectives are needed.

Math folded per core (N_loc=1024, D=1024, E=8, O=1024, H=256):
    h      = relu(x @ W_g1.T + b_g1)                 # gating MLP, bf16 matmuls
    gates  = softmax(h @ W_g2.T + b_g2)              # fp32 softmax
    out    = sum_e gates[:,e] * (x @ W_e[e].T) + gates @ b_e

Schedule (measured ~258-260 us/core on hardware; PE issue floor for the
1024 expert matmuls alone is ~221 us):
  - inputs stream as per-dk (gx, w0) pairs on the Sync HW-DGE portal, in
    exactly the order the interleaved gating/expert-0 matmuls consume them
    (dma_start BLOCKS its issuing engine on the queue ring slot, so bulk
    DMAs never ride compute engines; small tensors use the GpSimd SW DGE);
  - gating L1 runs as two half-token passes: pass 1 interleaves with
    expert 0's first three tile-groups chunk-by-chunk while DMA is the
    bottleneck, pass 2 (no new data) fills the w0 tail-arrival window;
  - expert tile-groups accumulate over D into [P,1024] two-bank psums
    (oh-paired 512-wide matmuls); gate weighting is ACT mul + DVE add;
  - the bias rides a tiny K=8 matmul (gates.T stationary) inside expert
    2's stream; gates.T transposes hide inside expert 1's stream;
  - the final (expert,tile) runs four independent quarter-psums so each
    quarter's mul/add/DMA overlaps the next quarter's matmuls;
  - output is DMA'd bf16 (halved tail traffic) and upcast on host.

All matmul operands are bf16 (host-cast); accumulation fp32.
"""

import numpy as np
import ml_dtypes

import concourse.bass as bass
import concourse.mybir as mybir
import concourse.tile as tile
from concourse.bass_utils import run_bass_kernel_spmd

N, D, E, O, H = 8192, 1024, 8, 1024, 256
NCORES = 8
NLOC = N // NCORES          # 1024 rows per core
P = 128                     # partitions
NT = NLOC // P              # 8 n-tiles
DK = D // P                 # 8 contraction tiles
H2 = H // P                 # 2 h-tiles
NWARM = 10
BF16 = mybir.dt.bfloat16
F32 = mybir.dt.float32
BF = ml_dtypes.bfloat16


def legalize_single_wait(nc, max_waits=1):
    """This walrus build rejects instructions carrying more than one sync
    wait. Split each multi-wait instruction: excess waits move onto fresh
    same-engine NoOps inserted immediately before it (identical semantics:
    the engine stalls at the same program point on every semaphore)."""
    for f in nc.m.functions:
        for blk in f.blocks:
            insts = list(blk.instructions)
            if all(
                (i.sync_info is None or len(i.sync_info.on_wait) <= max_waits)
                for i in insts
            ):
                continue
            new = []
            for inst in insts:
                si = inst.sync_info
                if si is not None and len(si.on_wait) > max_waits:
                    waits = list(si.on_wait)
                    for k, w in enumerate(waits[:-max_waits]):
                        nop = mybir.InstNoOp(name=f"{inst.name}-w{k}")
                        nop.engine = inst.engine
                        nop.sync_info = mybir.SyncInfo(on_wait=[w], on_update=[])
                        new.append(nop)
                    si.on_wait = waits[-max_waits:]
                new.append(inst)
            blk.instructions = new
    return nc


def build_moe():
    nc = bass.Bass(target_bir_lowering=False)
    # gx packs [wg1t | xT] along the free dim: one DMA per dk-chunk feeds
    # both the gating matmuls and the expert stationaries (descriptor
    # generation on the Sync engine is serial, ~0.65us per dma_start —
    # fewer, fatter transfers get the critical prefix in sooner)
    gx = nc.dram_tensor("gx", [D, H + NLOC], BF16, kind="ExternalInput")
    wt = nc.dram_tensor("wt", [E, D, O], BF16, kind="ExternalInput")
    wg2t = nc.dram_tensor("wg2t", [H, E], BF16, kind="ExternalInput")
    bg1 = nc.dram_tensor("bg1", [H], F32, kind="ExternalInput")
    bg2 = nc.dram_tensor("bg2", [E], BF16, kind="ExternalInput")
    be = nc.dram_tensor("be", [E, O], BF16, kind="ExternalInput")
    ident = nc.dram_tensor("ident", [P, P], F32, kind="ExternalInput")
    out = nc.dram_tensor("out", [NLOC, O], BF16, kind="ExternalOutput")

    with tile.TileContext(nc) as tc:
        with (
            tc.tile_pool(name="const", bufs=1) as constp,
            tc.tile_pool(name="wpool", bufs=3) as wpool,
            tc.tile_pool(name="work", bufs=3) as workp,
            tc.tile_pool(name="gate_ps", bufs=2, space="PSUM") as gatep,
            tc.tile_pool(name="wide_ps", bufs=3, space="PSUM") as widep,
        ):
            # ---- PE warm-up: dummy matmuls on memset tiles (no DMA deps)
            # keep the PE busy while the first transfers land, so the HAM
            # clock-gate ramps before real work arrives ----
            warm_a = constp.tile([P, P], BF16, tag="warm_a")
            nc.vector.memset(warm_a, 0.0)
            warm_b = constp.tile([P, 512], BF16, tag="warm_b")
            nc.vector.memset(warm_b, 0.0)
            for i in range(NWARM):
                wpsum = gatep.tile([P, 512], F32, tag="g", name=f"warm{i}")
                nc.tensor.matmul(wpsum, warm_a, warm_b, start=True, stop=True)

            # ---- resident inputs ----
            gx_sb = [
                constp.tile([P, H + NLOC], BF16, tag=f"gx{dk}", name=f"gx{dk}")
                for dk in range(DK)
            ]
            wg1t_sb = [t[:, 0:H] for t in gx_sb]
            xT_sb = [t[:, H : H + NLOC] for t in gx_sb]
            wt0_r = wt[0].rearrange("(dk p) o -> p dk o", p=P)
            w0_sb = wpool.tile([P, DK, O], BF16, tag="wh", name="w0_sb")
            # per-dk (gx, w0) pairs all on the Sync portal — dma_start is a
            # BLOCKING instruction on its issuing engine (it waits for the
            # target queue's ring slot), so compute engines must never issue
            # bulk DMAs.  Pair order = the interleaved consumption order.
            for dk in range(DK):
                nc.sync.dma_start(
                    out=gx_sb[dk], in_=gx[dk * P : (dk + 1) * P, :]
                )
                nc.sync.dma_start(out=w0_sb[:, dk, :], in_=wt0_r[:, dk, :])
            # prefetch expert 1 immediately after the critical stream
            wt1_r = wt[1].rearrange("(dk p) o -> p dk o", p=P)
            w1_sb = wpool.tile([P, DK, O], BF16, tag="wh", name="w1_sb")
            nc.sync.dma_start(out=w1_sb, in_=wt1_r[:, :, :])
            w_sb = {0: w0_sb, 1: w1_sb}
            # small tensors ride the (otherwise idle) GpSimd software DGE
            bg1_sb = constp.tile([P, H2], F32, tag="bg1")
            nc.gpsimd.dma_start(out=bg1_sb, in_=bg1.rearrange("(h2 p) -> p h2", p=P))
            wg2t_sb = constp.tile([P, H2, E], BF16, tag="wg2t")
            nc.gpsimd.dma_start(
                out=wg2t_sb, in_=wg2t.rearrange("(h2 p) e -> p h2 e", p=P)
            )
            bg2_sb = constp.tile([1, E], BF16, tag="bg2")
            nc.gpsimd.dma_start(out=bg2_sb, in_=bg2[:])
            ones_sb = constp.tile([1, P], BF16, tag="ones")
            nc.vector.memset(ones_sb, 1.0)

            gates_sb = []
            gatesT_sb = []
            acc_sb = [
                constp.tile([P, O], F32, tag=f"acc{nt}", name=f"acc{nt}")
                for nt in range(NT)
            ]
            out_bf = [
                constp.tile([P, O], BF16, tag=f"obf{nt}", name=f"obf{nt}")
                for nt in range(NT)
            ]

            def expert_mm(e, nt):
                psum = widep.tile([P, O], F32, tag="mm", name=f"ps_e{e}_{nt}")
                for dk in range(DK):
                    for oh in range(2):
                        nc.tensor.matmul(
                            psum[:, oh * 512 : (oh + 1) * 512],
                            xT_sb[dk][:, nt * P : (nt + 1) * P],
                            w_sb[e][:, dk, oh * 512 : (oh + 1) * 512],
                            start=(dk == 0),
                            stop=(dk == DK - 1),
                        )
                return psum

            def expert_post(e, nt, psum):
                acc = acc_sb[nt]
                if e == 0:
                    nc.scalar.mul(acc, psum, gates_sb[nt][:, e : e + 1])
                elif e < E - 1:
                    tmp = workp.tile([P, O], F32, tag="tmp", name="tmp")
                    nc.scalar.mul(tmp, psum, gates_sb[nt][:, e : e + 1])
                    nc.vector.tensor_add(acc, acc, tmp)
                else:
                    # last expert: fine-grained post chain so the output
                    # DMAs start as early as possible
                    for oh in range(2):
                        sl = slice(oh * 512, (oh + 1) * 512)
                        tmp = workp.tile([P, 512], F32, tag="tmph", name="tmph")
                        nc.scalar.mul(tmp, psum[:, sl], gates_sb[nt][:, e : e + 1])
                        nc.vector.tensor_add(out_bf[nt][:, sl], acc[:, sl], tmp)
                        nc.sync.dma_start(
                            out=out[nt * P : (nt + 1) * P, sl],
                            in_=out_bf[nt][:, sl],
                        )

            def last_tile_group():
                # final (expert, tile): four independent quarter-psums so
                # each quarter's post-chain (mul/add/DMA) overlaps the next
                # quarter's matmuls — only the last quarter's chain remains
                # on the critical tail after the very last matmul
                e, nt = E - 1, NT - 1
                for q in range(4):
                    sl = slice(q * 256, (q + 1) * 256)
                    ph = gatep.tile([P, 256], F32, tag="g", name=f"pslast{q}")
                    for dk in range(DK):
                        nc.tensor.matmul(
                            ph,
                            xT_sb[dk][:, nt * P : (nt + 1) * P],
                            w_sb[e][:, dk, sl],
                            start=(dk == 0),
                            stop=(dk == DK - 1),
                        )
                    tmp = workp.tile([P, 256], F32, tag="tmpq", name="tmpq")
                    nc.scalar.mul(tmp, ph, gates_sb[nt][:, e : e + 1])
                    nc.vector.tensor_add(
                        out_bf[nt][:, sl], acc_sb[nt][:, sl], tmp
                    )
                    nc.sync.dma_start(
                        out=out[nt * P : (nt + 1) * P, sl],
                        in_=out_bf[nt][:, sl],
                    )

            def expert_group(e, nt):
                expert_post(e, nt, expert_mm(e, nt))

            # ---- startup interleave: gating L1 (first token-half) and expert
            # 0's first three tile-groups consume each (gx,w0) chunk-pair the
            # moment it lands, keeping the PE fed while DMA is the
            # bottleneck.  PSUM: gating 2x[P,512] + 3 wide expert tiles ----
            hT_sb = [
                constp.tile([P, NLOC], BF16, tag=f"hT{h2}", name=f"hT{h2}")
                for h2 in range(H2)
            ]
            psum_g0 = [
                gatep.tile([P, 512], F32, tag="g", name=f"pg{h2}a")
                for h2 in range(H2)
            ]
            early_psums = [
                widep.tile([P, O], F32, tag="mm", name=f"ps_e0_{nt}")
                for nt in range(3)
            ]
            def e0_mm(dk):
                for nt in range(3):
                    for oh in range(2):
                        nc.tensor.matmul(
                            early_psums[nt][:, oh * 512 : (oh + 1) * 512],
                            xT_sb[dk][:, nt * P : (nt + 1) * P],
                            w_sb[0][:, dk, oh * 512 : (oh + 1) * 512],
                            start=(dk == 0),
                            stop=(dk == DK - 1),
                        )

            for dk in range(DK):
                for h2 in range(H2):
                    nc.tensor.matmul(
                        psum_g0[h2],
                        wg1t_sb[dk][:, h2 * P : (h2 + 1) * P],
                        xT_sb[dk][:, 0:512],
                        start=(dk == 0),
                        stop=(dk == DK - 1),
                    )
                # expert-0's last two chunks are deferred past the second
                # gating pass: gating pass 1 completes as soon as gx lands,
                # and pass 2 (no new data) fills the w0 tail-arrival window
                if dk < DK - 2:
                    e0_mm(dk)
            for h2 in range(H2):
                nc.scalar.activation(
                    out=hT_sb[h2][:, 0:512],
                    in_=psum_g0[h2],
                    func=mybir.ActivationFunctionType.Relu,
                    bias=bg1_sb[:, h2 : h2 + 1],
                )
            # second token-half gating pass (xT fully resident by now)
            psum_g1 = [
                gatep.tile([P, 512], F32, tag="g", name=f"pg{h2}b")
                for h2 in range(H2)
            ]
            for dk in range(DK):
                for h2 in range(H2):
                    nc.tensor.matmul(
                        psum_g1[h2],
                        wg1t_sb[dk][:, h2 * P : (h2 + 1) * P],
                        xT_sb[dk][:, 512:1024],
                        start=(dk == 0),
                        stop=(dk == DK - 1),
                    )
            e0_mm(DK - 2)
            e0_mm(DK - 1)
            for h2 in range(H2):
                nc.scalar.activation(
                    out=hT_sb[h2][:, 512:1024],
                    in_=psum_g1[h2],
                    func=mybir.ActivationFunctionType.Relu,
                    bias=bg1_sb[:, h2 : h2 + 1],
                )

            # ---- gating: logits -> softmax -> gates, vectorized over all
            # tiles in one [P, NT*E] psum.  Logits here are bounded (~±2),
            # so exp needs no max-subtraction in fp32; a single exp + one
            # grouped reduce replaces 8 serialized per-tile chains ----
            for nt in range(NT):
                psum_l = gatep.tile([P, E], F32, tag="g", name=f"psl{nt}")
                for h2 in range(H2):
                    nc.tensor.matmul(
                        psum_l,
                        hT_sb[h2][:, nt * P : (nt + 1) * P],
                        wg2t_sb[:, h2, :],
                        start=(h2 == 0),
                        stop=False,
                    )
                nc.tensor.matmul(psum_l, ones_sb, bg2_sb, start=False, stop=True)

                negmax = workp.tile([P, 1], F32, tag="negmax")
                nc.vector.reduce_max(
                    negmax, psum_l, axis=mybir.AxisListType.X, negate=True
                )
                gates = constp.tile([P, E], F32, tag=f"gates{nt}", name=f"gates{nt}")
                sumexp = workp.tile([P, 1], F32, tag="sumexp")
                nc.scalar.activation(
                    out=gates,
                    in_=psum_l,
                    func=mybir.ActivationFunctionType.Exp,
                    bias=negmax,
                    accum_out=sumexp,
                )
                rsum = workp.tile([P, 1], F32, tag="rsum")
                nc.vector.reciprocal(rsum, sumexp)
                nc.vector.tensor_scalar_mul(gates, gates, rsum)
                gates_sb.append(gates)

            # late, non-critical constants on the GpSimd software DGE
            be_sb = constp.tile([E, O], BF16, tag="be")
            nc.gpsimd.dma_start(out=be_sb, in_=be[:, :])
            ident_sb = constp.tile([P, P], F32, tag="ident")
            nc.gpsimd.dma_start(out=ident_sb, in_=ident[:, :])

            # ---- expert-0 gate weighting + remaining tile-groups ----
            for nt in range(3):
                expert_post(0, nt, early_psums[nt])
            for nt in range(3, NT):
                expert_group(0, nt)

            # ---- main loop: stream experts, accumulate gate-weighted GEMM ----
            for e in range(1, E):
                if e + 1 < E:
                    wtn_r = wt[e + 1].rearrange("(dk p) o -> p dk o", p=P)
                    wn = wpool.tile([P, DK, O], BF16, tag="wh", name=f"w{e+1}_sb")
                    # split across two queues: one queue sustains only
                    # ~115 GB/s, which cuts arrival margins too close
                    nc.sync.dma_start(out=wn[:, 0:4, :], in_=wtn_r[:, 0:4, :])
                    nc.sync.dma_start(out=wn[:, 4:8, :], in_=wtn_r[:, 4:8, :])
                    w_sb[e + 1] = wn
                for nt in range(NT):
                    if e == E - 1 and nt == NT - 1:
                        last_tile_group()
                        continue
                    expert_group(e, nt)
                    if e == 1:
                        # gates.T via PE transpose, interleaved with expert
                        # 1's stream (softmax chain is long done: no stalls)
                        psum_t = gatep.tile([E, P], F32, tag="g", name=f"pst{nt}")
                        nc.tensor.transpose(psum_t, gates_sb[nt], ident_sb)
                        gatesT = constp.tile(
                            [E, P], BF16, tag=f"gatesT{nt}", name=f"gatesT{nt}"
                        )
                        nc.scalar.copy(out=gatesT, in_=psum_t)
                        gatesT_sb.append(gatesT)
                    if e == 2:
                        # bias matmul: psum_b = gates.T @ b_e, added into acc;
                        # half-width psums from the gating pool so the wide
                        # expert-psum rotation keeps its full depth
                        for oh in range(2):
                            sl = slice(oh * 512, (oh + 1) * 512)
                            psum_b = gatep.tile(
                                [P, 512], F32, tag="g", name="psum_b"
                            )
                            nc.tensor.matmul(
                                psum_b, gatesT_sb[nt], be_sb[:, sl],
                                start=True, stop=True,
                            )
                            nc.vector.tensor_add(
                                acc_sb[nt][:, sl], acc_sb[nt][:, sl], psum_b
                            )

    legalize_single_wait(nc)
    return nc


_NC_CACHE = {}


def _get_nc():
    if "nc" not in _NC_CACHE:
        _NC_CACHE["nc"] = build_moe()
    return _NC_CACHE["nc"]


def make_in_maps(x, W_e, b_e, W_g1, b_g1, W_g2, b_g2):
    x = np.asarray(x, dtype=np.float32)
    wt = np.ascontiguousarray(
        np.asarray(W_e, dtype=np.float32).transpose(0, 2, 1)
    ).astype(BF)
    wg1t = np.ascontiguousarray(np.asarray(W_g1, dtype=np.float32).T).astype(BF)
    wg2t = np.ascontiguousarray(np.asarray(W_g2, dtype=np.float32).T).astype(BF)
    bg1 = np.asarray(b_g1, dtype=np.float32)
    bg2 = np.asarray(b_g2, dtype=np.float32).astype(BF)
    be = np.asarray(b_e, dtype=np.float32).astype(BF)
    xb = x.astype(BF)
    ident_np = np.eye(P, dtype=np.float32)
    in_maps = []
    for c in range(NCORES):
        xT_c = np.ascontiguousarray(xb[c * NLOC : (c + 1) * NLOC, :].T)
        gx_c = np.ascontiguousarray(np.concatenate([wg1t, xT_c], axis=1))
        in_maps.append(
            {
                "gx": gx_c,
                "wt": wt,
                "wg2t": wg2t,
                "bg1": bg1,
                "bg2": bg2,
                "be": be,
                "ident": ident_np,
            }
        )
    return in_maps


def kernel(x, W_e, b_e, W_g1, b_g1, W_g2, b_g2, **run_kwargs):
    nc = _get_nc()
    in_maps = make_in_maps(x, W_e, b_e, W_g1, b_g1, W_g2, b_g2)
    res = run_bass_kernel_spmd(nc, in_maps, core_ids=list(range(NCORES)), **run_kwargs)
    out = np.concatenate(
        [res.results[c]["out"].astype(np.float32) for c in range(NCORES)], axis=0
    )
    if run_kwargs:
        kernel.last_results = res
    return out


if __name__ == "__main__":
    rng = np.random.default_rng(0)
    s = 1.0 / np.sqrt(D)
    sh = 1.0 / np.sqrt(H)
    inputs = {
        "x": rng.standard_normal((N, D), dtype=np.float32),
        "W_e": rng.uniform(-s, s, (E, O, D)).astype(np.float32),
        "b_e": rng.uniform(-s, s, (E, O)).astype(np.float32),
        "W_g1": rng.uniform(-s, s, (H, D)).astype(np.float32),
        "b_g1": rng.uniform(-s, s, (H,)).astype(np.float32),
        "W_g2": rng.uniform(-sh, sh, (E, H)).astype(np.float32),
        "b_g2": rng.uniform(-sh, sh, (E,)).astype(np.float32),
    }
    out = kernel(**inputs)
    print("out", out.shape, out.dtype, float(np.abs(out).max()))


# revision 58
# speedup vs baseline: 1.0049x; 1.0049x over previous
"""Dense MoE (all-experts, gate-weighted sum) on 8 Trainium2 NeuronCores.

Sharding: pure data-parallel over the token axis N (8192 -> 1024 rows/core);
every core holds all 8 experts, so no collectives are needed.

Math folded per core (N_loc=1024, D=1024, E=8, O=1024, H=256):
    h      = relu(x @ W_g1.T + b_g1)                 # gating MLP, bf16 matmuls
    gates  = softmax(h @ W_g2.T + b_g2)              # fp32 softmax
    out    = sum_e gates[:,e] * (x @ W_e[e].T) + gates @ b_e

Schedule (measured ~258-260 us/core on hardware; PE issue floor for the
1024 expert matmuls alone is ~221 us):
  - inputs stream as per-dk (gx, w0) pairs on the Sync HW-DGE portal, in
    exactly the order the interleaved gating/expert-0 matmuls consume them
    (dma_start BLOCKS its issuing engine on the queue ring slot, so bulk
    DMAs never ride compute engines; small tensors use the GpSimd SW DGE);
  - gating L1 runs as two half-token passes: pass 1 interleaves with
    expert 0's first three tile-groups chunk-by-chunk while DMA is the
    bottleneck, pass 2 (no new data) fills the w0 tail-arrival window;
  - expert tile-groups accumulate over D into [P,1024] two-bank psums
    (oh-paired 512-wide matmuls); gate weighting is ACT mul + DVE add;
  - the bias rides a tiny K=8 matmul (gates.T stationary) inside expert
    2's stream; gates.T transposes hide inside expert 1's stream;
  - the final (expert,tile) runs four independent quarter-psums so each
    quarter's mul/add/DMA overlaps the next quarter's matmuls;
  - output is DMA'd bf16 (halved tail traffic) and upcast on host.

All matmul operands are bf16 (host-cast); accumulation fp32.
"""

import numpy as np
import ml_dtypes

import concourse.bass as bass
import concourse.mybir as mybir
import concourse.tile as tile
from concourse.bass_utils import run_bass_kernel_spmd

N, D, E, O, H = 8192, 1024, 8, 1024, 256
NCORES = 8
NLOC = N // NCORES          # 1024 rows per core
P = 128                     # partitions
NT = NLOC // P              # 8 n-tiles
DK = D // P                 # 8 contraction tiles
H2 = H // P                 # 2 h-tiles
NWARM = 10
BF16 = mybir.dt.bfloat16
F32 = mybir.dt.float32
BF = ml_dtypes.bfloat16


def legalize_single_wait(nc, max_waits=1):
    """This walrus build rejects instructions carrying more than one sync
    wait. Split each multi-wait instruction: excess waits move onto fresh
    same-engine NoOps inserted immediately before it (identical semantics:
    the engine stalls at the same program point on every semaphore)."""
    for f in nc.m.functions:
        for blk in f.blocks:
            insts = list(blk.instructions)
            if all(
                (i.sync_info is None or len(i.sync_info.on_wait) <= max_waits)
                for i in insts
            ):
                continue
            new = []
            for inst in insts:
                si = inst.sync_info
                if si is not None and len(si.on_wait) > max_waits:
                    waits = list(si.on_wait)
                    for k, w in enumerate(waits[:-max_waits]):
                        nop = mybir.InstNoOp(name=f"{inst.name}-w{k}")
                        nop.engine = inst.engine
                        nop.sync_info = mybir.SyncInfo(on_wait=[w], on_update=[])
                        new.append(nop)
                    si.on_wait = waits[-max_waits:]
                new.append(inst)
            blk.instructions = new
    return nc


def build_moe():
    nc = bass.Bass(target_bir_lowering=False)
    # gx packs [wg1t | xT] along the free dim: one DMA per dk-chunk feeds
    # both the gating matmuls and the expert stationaries (descriptor
    # generation on the Sync engine is serial, ~0.65us per dma_start —
    # fewer, fatter transfers get the critical prefix in sooner)
    gx = nc.dram_tensor("gx", [D, H + NLOC], BF16, kind="ExternalInput")
    wt = nc.dram_tensor("wt", [E, D, O], BF16, kind="ExternalInput")
    wg2t = nc.dram_tensor("wg2t", [H, E], BF16, kind="ExternalInput")
    bg1 = nc.dram_tensor("bg1", [H], F32, kind="ExternalInput")
    bg2 = nc.dram_tensor("bg2", [E], BF16, kind="ExternalInput")
    be = nc.dram_tensor("be", [E, O], BF16, kind="ExternalInput")
    ident = nc.dram_tensor("ident", [P, P], F32, kind="ExternalInput")
    out = nc.dram_tensor("out", [NLOC, O], BF16, kind="ExternalOutput")

    with tile.TileContext(nc) as tc:
        with (
            tc.tile_pool(name="const", bufs=1) as constp,
            tc.tile_pool(name="wpool", bufs=3) as wpool,
            tc.tile_pool(name="work", bufs=3) as workp,
            tc.tile_pool(name="gate_ps", bufs=2, space="PSUM") as gatep,
            tc.tile_pool(name="wide_ps", bufs=3, space="PSUM") as widep,
        ):
            # ---- PE warm-up: dummy matmuls on memset tiles (no DMA deps)
            # keep the PE busy while the first transfers land, so the HAM
            # clock-gate ramps before real work arrives ----
            warm_a = constp.tile([P, P], BF16, tag="warm_a")
            nc.vector.memset(warm_a, 0.0)
            warm_b = constp.tile([P, 512], BF16, tag="warm_b")
            nc.vector.memset(warm_b, 0.0)
            for i in range(NWARM):
                wpsum = gatep.tile([P, 512], F32, tag="g", name=f"warm{i}")
                nc.tensor.matmul(wpsum, warm_a, warm_b, start=True, stop=True)

            # ---- resident inputs ----
            gx_sb = [
                constp.tile([P, H + NLOC], BF16, tag=f"gx{dk}", name=f"gx{dk}")
                for dk in range(DK)
            ]
            wg1t_sb = [t[:, 0:H] for t in gx_sb]
            xT_sb = [t[:, H : H + NLOC] for t in gx_sb]
            wt0_r = wt[0].rearrange("(dk p) o -> p dk o", p=P)
            w0_sb = wpool.tile([P, DK, O], BF16, tag="wh", name="w0_sb")
            # per-dk (gx, w0) pairs all on the Sync portal — dma_start is a
            # BLOCKING instruction on its issuing engine (it waits for the
            # target queue's ring slot), so compute engines must never issue
            # bulk DMAs.  Pair order = the interleaved consumption order.
            for dk in range(DK):
                nc.sync.dma_start(
                    out=gx_sb[dk], in_=gx[dk * P : (dk + 1) * P, :]
                )
                nc.sync.dma_start(out=w0_sb[:, dk, :], in_=wt0_r[:, dk, :])
            # prefetch expert 1 immediately after the critical stream
            wt1_r = wt[1].rearrange("(dk p) o -> p dk o", p=P)
            w1_sb = wpool.tile([P, DK, O], BF16, tag="wh", name="w1_sb")
            nc.sync.dma_start(out=w1_sb, in_=wt1_r[:, :, :])
            w_sb = {0: w0_sb, 1: w1_sb}
            # small tensors ride the (otherwise idle) GpSimd software DGE
            bg1_sb = constp.tile([P, H2], F32, tag="bg1")
            nc.gpsimd.dma_start(out=bg1_sb, in_=bg1.rearrange("(h2 p) -> p h2", p=P))
            wg2t_sb = constp.tile([P, H2, E], BF16, tag="wg2t")
            nc.gpsimd.dma_start(
                out=wg2t_sb, in_=wg2t.rearrange("(h2 p) e -> p h2 e", p=P)
            )
            bg2_sb = constp.tile([1, E], BF16, tag="bg2")
            nc.gpsimd.dma_start(out=bg2_sb, in_=bg2[:])
            ones_sb = constp.tile([1, P], BF16, tag="ones")
            nc.vector.memset(ones_sb, 1.0)

            gates_sb = []
            gatesT_sb = []
            acc_sb = [
                constp.tile([P, O], F32, tag=f"acc{nt}", name=f"acc{nt}")
                for nt in range(NT)
            ]
            out_bf = [
                constp.tile([P, O], BF16, tag=f"obf{nt}", name=f"obf{nt}")
                for nt in range(NT)
            ]

            def expert_mm(e, nt):
                psum = widep.tile([P, O], F32, tag="mm", name=f"ps_e{e}_{nt}")
                for dk in range(DK):
                    for oh in range(2):
                        nc.tensor.matmul(
                            psum[:, oh * 512 : (oh + 1) * 512],
                            xT_sb[dk][:, nt * P : (nt + 1) * P],
                            w_sb[e][:, dk, oh * 512 : (oh + 1) * 512],
                            start=(dk == 0),
                            stop=(dk == DK - 1),
                        )
                return psum

            def expert_post(e, nt, psum):
                acc = acc_sb[nt]
                if e == 0:
                    nc.scalar.mul(acc, psum, gates_sb[nt][:, e : e + 1])
                elif e < E - 1:
                    tmp = workp.tile([P, O], F32, tag="tmp", name="tmp")
                    nc.scalar.mul(tmp, psum, gates_sb[nt][:, e : e + 1])
                    nc.vector.tensor_add(acc, acc, tmp)
                else:
                    # last expert: fine-grained post chain so the output
                    # DMAs start as early as possible
                    for oh in range(2):
                        sl = slice(oh * 512, (oh + 1) * 512)
                        tmp = workp.tile([P, 512], F32, tag="tmph", name="tmph")
                        nc.scalar.mul(tmp, psum[:, sl], gates_sb[nt][:, e : e + 1])
                        nc.vector.tensor_add(out_bf[nt][:, sl], acc[:, sl], tmp)
                        nc.sync.dma_start(
                            out=out[nt * P : (nt + 1) * P, sl],
                            in_=out_bf[nt][:, sl],
                        )

            def last_tile_group():
                # final (expert, tile): four independent quarter-psums so
                # each quarter's post-chain (mul/add/DMA) overlaps the next
                # quarter's matmuls — only the last quarter's chain remains
                # on the critical tail after the very last matmul
                e, nt = E - 1, NT - 1
                for q in range(4):
                    sl = slice(q * 256, (q + 1) * 256)
                    ph = gatep.tile([P, 256], F32, tag="g", name=f"pslast{q}")
                    for dk in range(DK):
                        nc.tensor.matmul(
                            ph,
                            xT_sb[dk][:, nt * P : (nt + 1) * P],
                            w_sb[e][:, dk, sl],
                            start=(dk == 0),
                            stop=(dk == DK - 1),
                        )
                    tmp = workp.tile([P, 256], F32, tag="tmpq", name="tmpq")
                    nc.scalar.mul(tmp, ph, gates_sb[nt][:, e : e + 1])
                    nc.vector.tensor_add(
                        out_bf[nt][:, sl], acc_sb[nt][:, sl], tmp
                    )
                    nc.sync.dma_start(
                        out=out[nt * P : (nt + 1) * P, sl],
                        in_=out_bf[nt][:, sl],
                    )

            def expert_group(e, nt):
                expert_post(e, nt, expert_mm(e, nt))

            # ---- startup interleave: gating L1 (first token-half) and expert
            # 0's first three tile-groups consume each (gx,w0) chunk-pair the
            # moment it lands, keeping the PE fed while DMA is the
            # bottleneck.  PSUM: gating 2x[P,512] + 3 wide expert tiles ----
            hT_sb = [
                constp.tile([P, NLOC], BF16, tag=f"hT{h2}", name=f"hT{h2}")
                for h2 in range(H2)
            ]
            psum_g0 = [
                gatep.tile([P, 512], F32, tag="g", name=f"pg{h2}a")
                for h2 in range(H2)
            ]
            early_psums = [
                widep.tile([P, O], F32, tag="mm", name=f"ps_e0_{nt}")
                for nt in range(3)
            ]
            def e0_mm(dk):
                for nt in range(3):
                    for oh in range(2):
                        nc.tensor.matmul(
                            early_psums[nt][:, oh * 512 : (oh + 1) * 512],
                            xT_sb[dk][:, nt * P : (nt + 1) * P],
                            w_sb[0][:, dk, oh * 512 : (oh + 1) * 512],
                            start=(dk == 0),
                            stop=(dk == DK - 1),
                        )

            for dk in range(DK):
                for h2 in range(H2):
                    nc.tensor.matmul(
                        psum_g0[h2],
                        wg1t_sb[dk][:, h2 * P : (h2 + 1) * P],
                        xT_sb[dk][:, 0:512],
                        start=(dk == 0),
                        stop=(dk == DK - 1),
                    )
                # expert-0's last two chunks are deferred past the second
                # gating pass: gating pass 1 completes as soon as gx lands,
                # and pass 2 (no new data) fills the w0 tail-arrival window
                if dk < DK - 2:
                    e0_mm(dk)
            for h2 in range(H2):
                nc.scalar.activation(
                    out=hT_sb[h2][:, 0:512],
                    in_=psum_g0[h2],
                    func=mybir.ActivationFunctionType.Relu,
                    bias=bg1_sb[:, h2 : h2 + 1],
                )
            # second token-half gating pass (xT fully resident by now)
            psum_g1 = [
                gatep.tile([P, 512], F32, tag="g", name=f"pg{h2}b")
                for h2 in range(H2)
            ]
            for dk in range(DK):
                for h2 in range(H2):
                    nc.tensor.matmul(
                        psum_g1[h2],
                        wg1t_sb[dk][:, h2 * P : (h2 + 1) * P],
                        xT_sb[dk][:, 512:1024],
                        start=(dk == 0),
                        stop=(dk == DK - 1),
                    )
            e0_mm(DK - 2)
            e0_mm(DK - 1)
            for h2 in range(H2):
                nc.scalar.activation(
                    out=hT_sb[h2][:, 512:1024],
                    in_=psum_g1[h2],
                    func=mybir.ActivationFunctionType.Relu,
                    bias=bg1_sb[:, h2 : h2 + 1],
                )

            # ---- gating: logits -> softmax -> gates, vectorized over all
            # tiles in one [P, NT*E] psum.  Logits here are bounded (~±2),
            # so exp needs no max-subtraction in fp32; a single exp + one
            # grouped reduce replaces 8 serialized per-tile chains ----
            for nt in range(NT):
                psum_l = gatep.tile([P, E], F32, tag="g", name=f"psl{nt}")
                for h2 in range(H2):
                    nc.tensor.matmul(
                        psum_l,
                        hT_sb[h2][:, nt * P : (nt + 1) * P],
                        wg2t_sb[:, h2, :],
                        start=(h2 == 0),
                        stop=False,
                    )
                nc.tensor.matmul(psum_l, ones_sb, bg2_sb, start=False, stop=True)

                negmax = workp.tile([P, 1], F32, tag="negmax")
                nc.vector.reduce_max(
                    negmax, psum_l, axis=mybir.AxisListType.X, negate=True
                )
                gates = constp.tile([P, E], F32, tag=f"gates{nt}", name=f"gates{nt}")
                sumexp = workp.tile([P, 1], F32, tag="sumexp")
                nc.scalar.activation(
                    out=gates,
                    in_=psum_l,
                    func=mybir.ActivationFunctionType.Exp,
                    bias=negmax,
                    accum_out=sumexp,
                )
                rsum = workp.tile([P, 1], F32, tag="rsum")
                nc.vector.reciprocal(rsum, sumexp)
                nc.vector.tensor_scalar_mul(gates, gates, rsum)
                gates_sb.append(gates)

            # late, non-critical constants on the GpSimd software DGE
            be_sb = constp.tile([E, O], BF16, tag="be")
            nc.gpsimd.dma_start(out=be_sb, in_=be[:, :])
            ident_sb = constp.tile([P, P], F32, tag="ident")
            nc.gpsimd.dma_start(out=ident_sb, in_=ident[:, :])

            # ---- expert-0 gate weighting + remaining tile-groups ----
            for nt in range(3):
                expert_post(0, nt, early_psums[nt])
            for nt in range(3, NT):
                expert_group(0, nt)

            # ---- main loop: stream experts, accumulate gate-weighted GEMM ----
            for e in range(1, E):
                if e + 1 < E:
                    wtn_r = wt[e + 1].rearrange("(dk p) o -> p dk o", p=P)
                    wn = wpool.tile([P, DK, O], BF16, tag="wh", name=f"w{e+1}_sb")
                    # split across two queues: one queue sustains only
                    # ~115 GB/s, which cuts arrival margins too close
                    nc.sync.dma_start(out=wn[:, 0:4, :], in_=wtn_r[:, 0:4, :])
                    nc.sync.dma_start(out=wn[:, 4:8, :], in_=wtn_r[:, 4:8, :])
                    w_sb[e + 1] = wn
                for nt in range(NT):
                    if e == E - 1 and nt == NT - 1:
                        last_tile_group()
                        continue
                    expert_group(e, nt)
                    if e == 1:
                        # gates.T via PE transpose, interleaved with expert
                        # 1's stream (softmax chain is long done: no stalls)
                        psum_t = gatep.tile([E, P], F32, tag="g", name=f"pst{nt}")
                        nc.tensor.transpose(psum_t, gates_sb[nt], ident_sb)
                        gatesT = constp.tile(
                            [E, P], BF16, tag=f"gatesT{nt}", name=f"gatesT{nt}"
                        )
                        nc.scalar.copy(out=gatesT, in_=psum_t)
                        gatesT_sb.append(gatesT)
                    if e == 2:
                        # bias matmul: psum_b = gates.T @ b_e, added into acc;
                        # half-width psums from the gating pool so the wide
                        # expert-psum rotation keeps its full depth
                        for oh in range(2):
                            sl = slice(oh * 512, (oh + 1) * 512)
                            psum_b = gatep.tile(
                                [P, 512], F32, tag="g", name="psum_b"
                            )
                            nc.tensor.matmul(
                                psum_b, gatesT_sb[nt], be_sb[:, sl],
                                start=True, stop=True,
                            )
                            nc.vector.tensor_add(
                                acc_sb[nt][:, sl], acc_sb[nt][:, sl], psum_b
                            )

    legalize_single_wait(nc)
    return nc


_NC_CACHE = {}


def _get_nc():
    if "nc" not in _NC_CACHE:
        _NC_CACHE["nc"] = build_moe()
    return _NC_CACHE["nc"]


def make_in_maps(x, W_e, b_e, W_g1, b_g1, W_g2, b_g2):
    x = np.asarray(x, dtype=np.float32)
    wt = np.ascontiguousarray(
        np.asarray(W_e, dtype=np.float32).transpose(0, 2, 1)
    ).astype(BF)
    wg1t = np.ascontiguousarray(np.asarray(W_g1, dtype=np.float32).T).astype(BF)
    wg2t = np.ascontiguousarray(np.asarray(W_g2, dtype=np.float32).T).astype(BF)
    bg1 = np.asarray(b_g1, dtype=np.float32)
    bg2 = np.asarray(b_g2, dtype=np.float32).astype(BF)
    be = np.asarray(b_e, dtype=np.float32).astype(BF)
    xb = x.astype(BF)
    ident_np = np.eye(P, dtype=np.float32)
    in_maps = []
    for c in range(NCORES):
        xT_c = np.ascontiguousarray(xb[c * NLOC : (c + 1) * NLOC, :].T)
        gx_c = np.ascontiguousarray(np.concatenate([wg1t, xT_c], axis=1))
        in_maps.append(
            {
                "gx": gx_c,
                "wt": wt,
                "wg2t": wg2t,
                "bg1": bg1,
                "bg2": bg2,
                "be": be,
                "ident": ident_np,
            }
        )
    return in_maps


def kernel(x, W_e, b_e, W_g1, b_g1, W_g2, b_g2, **run_kwargs):
    nc = _get_nc()
    in_maps = make_in_maps(x, W_e, b_e, W_g1, b_g1, W_g2, b_g2)
    res = run_bass_kernel_spmd(nc, in_maps, core_ids=list(range(NCORES)), **run_kwargs)
    out = np.concatenate(
        [res.results[c]["out"].astype(np.float32) for c in range(NCORES)], axis=0
    )
    if run_kwargs:
        kernel.last_results = res
    return out


if __name__ == "__main__":
    rng = np.random.default_rng(0)
    s = 1.0 / np.sqrt(D)
    sh = 1.0 / np.sqrt(H)
    inputs = {
        "x": rng.standard_normal((N, D), dtype=np.float32),
        "W_e": rng.uniform(-s, s, (E, O, D)).astype(np.float32),
        "b_e": rng.uniform(-s, s, (E, O)).astype(np.float32),
        "W_g1": rng.uniform(-s, s, (H, D)).astype(np.float32),
        "b_g1": rng.uniform(-s, s, (H,)).astype(np.float32),
        "W_g2": rng.uniform(-sh, sh, (E, H)).astype(np.float32),
        "b_g2": rng.uniform(-sh, sh, (E,)).astype(np.float32),
    }
    out = kernel(**inputs)
    print("out", out.shape, out.dtype, float(np.abs(out).max()))


# revision 60
# speedup vs baseline: 1.0100x; 1.0051x over previous
"""Dense MoE (all-experts, gate-weighted sum) on 8 Trainium2 NeuronCores.

Sharding: pure data-parallel over the token axis N (8192 -> 1024 rows/core);
every core holds all 8 experts, so no collectives are needed.

Math folded per core (N_loc=1024, D=1024, E=8, O=1024, H=256):
    h      = relu(x @ W_g1.T + b_g1)                 # gating MLP, bf16 matmuls
    gates  = softmax(h @ W_g2.T + b_g2)              # fp32 softmax
    out    = sum_e gates[:,e] * (x @ W_e[e].T) + gates @ b_e

Schedule (measured ~258-260 us/core on hardware; PE issue floor for the
1024 expert matmuls alone is ~221 us):
  - inputs stream as per-dk (gx, w0) pairs on the Sync HW-DGE portal, in
    exactly the order the interleaved gating/expert-0 matmuls consume them
    (dma_start BLOCKS its issuing engine on the queue ring slot, so bulk
    DMAs never ride compute engines; small tensors use the GpSimd SW DGE);
  - gating L1 runs as two half-token passes: pass 1 interleaves with
    expert 0's first three tile-groups chunk-by-chunk while DMA is the
    bottleneck, pass 2 (no new data) fills the w0 tail-arrival window;
  - expert tile-groups accumulate over D into [P,1024] two-bank psums
    (oh-paired 512-wide matmuls); gate weighting is ACT mul + DVE add;
  - the bias rides a tiny K=8 matmul (gates.T stationary) inside expert
    2's stream; gates.T transposes hide inside expert 1's stream;
  - the final (expert,tile) runs four independent quarter-psums so each
    quarter's mul/add/DMA overlaps the next quarter's matmuls;
  - output is DMA'd bf16 (halved tail traffic) and upcast on host.

All matmul operands are bf16 (host-cast); accumulation fp32.
"""

import numpy as np
import ml_dtypes

import concourse.bass as bass
import concourse.mybir as mybir
import concourse.tile as tile
from concourse.bass_utils import run_bass_kernel_spmd

N, D, E, O, H = 8192, 1024, 8, 1024, 256
NCORES = 8
NLOC = N // NCORES          # 1024 rows per core
P = 128                     # partitions
NT = NLOC // P              # 8 n-tiles
DK = D // P                 # 8 contraction tiles
H2 = H // P                 # 2 h-tiles
NWARM = 10
BF16 = mybir.dt.bfloat16
F32 = mybir.dt.float32
BF = ml_dtypes.bfloat16


def legalize_single_wait(nc, max_waits=1):
    """This walrus build rejects instructions carrying more than one sync
    wait. Split each multi-wait instruction: excess waits move onto fresh
    same-engine NoOps inserted immediately before it (identical semantics:
    the engine stalls at the same program point on every semaphore)."""
    for f in nc.m.functions:
        for blk in f.blocks:
            insts = list(blk.instructions)
            if all(
                (i.sync_info is None or len(i.sync_info.on_wait) <= max_waits)
                for i in insts
            ):
                continue
            new = []
            for inst in insts:
                si = inst.sync_info
                if si is not None and len(si.on_wait) > max_waits:
                    waits = list(si.on_wait)
                    for k, w in enumerate(waits[:-max_waits]):
                        nop = mybir.InstNoOp(name=f"{inst.name}-w{k}")
                        nop.engine = inst.engine
                        nop.sync_info = mybir.SyncInfo(on_wait=[w], on_update=[])
                        new.append(nop)
                    si.on_wait = waits[-max_waits:]
                new.append(inst)
            blk.instructions = new
    return nc


def build_moe():
    nc = bass.Bass(target_bir_lowering=False)
    # gx packs [wg1t | xT] along the free dim: one DMA per dk-chunk feeds
    # both the gating matmuls and the expert stationaries (descriptor
    # generation on the Sync engine is serial, ~0.65us per dma_start —
    # fewer, fatter transfers get the critical prefix in sooner)
    gx = nc.dram_tensor("gx", [D, H + NLOC], BF16, kind="ExternalInput")
    wt = nc.dram_tensor("wt", [E, D, O], BF16, kind="ExternalInput")
    wg2t = nc.dram_tensor("wg2t", [H, E], BF16, kind="ExternalInput")
    bg1 = nc.dram_tensor("bg1", [H], F32, kind="ExternalInput")
    bg2 = nc.dram_tensor("bg2", [E], BF16, kind="ExternalInput")
    be = nc.dram_tensor("be", [E, O], BF16, kind="ExternalInput")
    ident = nc.dram_tensor("ident", [P, P], F32, kind="ExternalInput")
    out = nc.dram_tensor("out", [NLOC, O], BF16, kind="ExternalOutput")

    with tile.TileContext(nc) as tc:
        with (
            tc.tile_pool(name="const", bufs=1) as constp,
            tc.tile_pool(name="wpool", bufs=3) as wpool,
            tc.tile_pool(name="work", bufs=3) as workp,
            tc.tile_pool(name="gate_ps", bufs=2, space="PSUM") as gatep,
            tc.tile_pool(name="wide_ps", bufs=3, space="PSUM") as widep,
        ):
            # ---- PE warm-up: dummy matmuls on memset tiles (no DMA deps)
            # keep the PE busy while the first transfers land, so the HAM
            # clock-gate ramps before real work arrives ----
            warm_a = constp.tile([P, P], BF16, tag="warm_a")
            nc.vector.memset(warm_a, 0.0)
            warm_b = constp.tile([P, 512], BF16, tag="warm_b")
            nc.vector.memset(warm_b, 0.0)
            for i in range(NWARM):
                wpsum = gatep.tile([P, 512], F32, tag="g", name=f"warm{i}")
                nc.tensor.matmul(wpsum, warm_a, warm_b, start=True, stop=True)

            # ---- resident inputs ----
            gx_sb = [
                constp.tile([P, H + NLOC], BF16, tag=f"gx{dk}", name=f"gx{dk}")
                for dk in range(DK)
            ]
            wg1t_sb = [t[:, 0:H] for t in gx_sb]
            xT_sb = [t[:, H : H + NLOC] for t in gx_sb]
            wt0_r = wt[0].rearrange("(dk p) o -> p dk o", p=P)
            w0_sb = wpool.tile([P, DK, O], BF16, tag="wh", name="w0_sb")
            # per-dk (gx, w0) pairs all on the Sync portal — dma_start is a
            # BLOCKING instruction on its issuing engine (it waits for the
            # target queue's ring slot), so compute engines must never issue
            # bulk DMAs.  Pair order = the interleaved consumption order.
            for dk in range(DK):
                nc.sync.dma_start(
                    out=gx_sb[dk], in_=gx[dk * P : (dk + 1) * P, :]
                )
                nc.sync.dma_start(out=w0_sb[:, dk, :], in_=wt0_r[:, dk, :])
            # prefetch expert 1 immediately after the critical stream
            wt1_r = wt[1].rearrange("(dk p) o -> p dk o", p=P)
            w1_sb = wpool.tile([P, DK, O], BF16, tag="wh", name="w1_sb")
            nc.sync.dma_start(out=w1_sb, in_=wt1_r[:, :, :])
            w_sb = {0: w0_sb, 1: w1_sb}
            # small tensors ride the (otherwise idle) GpSimd software DGE
            bg1_sb = constp.tile([P, H2], F32, tag="bg1")
            nc.gpsimd.dma_start(out=bg1_sb, in_=bg1.rearrange("(h2 p) -> p h2", p=P))
            wg2t_sb = constp.tile([P, H2, E], BF16, tag="wg2t")
            nc.gpsimd.dma_start(
                out=wg2t_sb, in_=wg2t.rearrange("(h2 p) e -> p h2 e", p=P)
            )
            bg2_sb = constp.tile([1, E], BF16, tag="bg2")
            nc.gpsimd.dma_start(out=bg2_sb, in_=bg2[:])
            ones_sb = constp.tile([1, P], BF16, tag="ones")
            nc.vector.memset(ones_sb, 1.0)

            gates_sb = []
            gatesT_sb = []
            acc_sb = [
                constp.tile([P, O], F32, tag=f"acc{nt}", name=f"acc{nt}")
                for nt in range(NT)
            ]
            out_bf = [
                constp.tile([P, O], BF16, tag=f"obf{nt}", name=f"obf{nt}")
                for nt in range(NT)
            ]

            def expert_mm(e, nt):
                psum = widep.tile([P, O], F32, tag="mm", name=f"ps_e{e}_{nt}")
                for dk in range(DK):
                    for oh in range(2):
                        nc.tensor.matmul(
                            psum[:, oh * 512 : (oh + 1) * 512],
                            xT_sb[dk][:, nt * P : (nt + 1) * P],
                            w_sb[e][:, dk, oh * 512 : (oh + 1) * 512],
                            start=(dk == 0),
                            stop=(dk == DK - 1),
                        )
                return psum

            def expert_post(e, nt, psum):
                acc = acc_sb[nt]
                if e == 0:
                    nc.scalar.mul(acc, psum, gates_sb[nt][:, e : e + 1])
                elif e < E - 1:
                    tmp = workp.tile([P, O], F32, tag="tmp", name="tmp")
                    nc.scalar.mul(tmp, psum, gates_sb[nt][:, e : e + 1])
                    nc.vector.tensor_add(acc, acc, tmp)
                else:
                    # last expert: fine-grained post chain so the output
                    # DMAs start as early as possible
                    for oh in range(2):
                        sl = slice(oh * 512, (oh + 1) * 512)
                        tmp = workp.tile([P, 512], F32, tag="tmph", name="tmph")
                        nc.scalar.mul(tmp, psum[:, sl], gates_sb[nt][:, e : e + 1])
                        nc.vector.tensor_add(out_bf[nt][:, sl], acc[:, sl], tmp)
                        nc.sync.dma_start(
                            out=out[nt * P : (nt + 1) * P, sl],
                            in_=out_bf[nt][:, sl],
                        )

            def last_tile_group():
                # final (expert, tile): four independent quarter-psums so
                # each quarter's post-chain (mul/add/DMA) overlaps the next
                # quarter's matmuls — only the last quarter's chain remains
                # on the critical tail after the very last matmul
                e, nt = E - 1, NT - 1
                for q in range(4):
                    sl = slice(q * 256, (q + 1) * 256)
                    ph = gatep.tile([P, 256], F32, tag="g", name=f"pslast{q}")
                    for dk in range(DK):
                        nc.tensor.matmul(
                            ph,
                            xT_sb[dk][:, nt * P : (nt + 1) * P],
                            w_sb[e][:, dk, sl],
                            start=(dk == 0),
                            stop=(dk == DK - 1),
                        )
                    tmp = workp.tile([P, 256], F32, tag="tmpq", name="tmpq")
                    nc.scalar.mul(tmp, ph, gates_sb[nt][:, e : e + 1])
                    nc.vector.tensor_add(
                        out_bf[nt][:, sl], acc_sb[nt][:, sl], tmp
                    )
                    nc.sync.dma_start(
                        out=out[nt * P : (nt + 1) * P, sl],
                        in_=out_bf[nt][:, sl],
                    )

            def expert_group(e, nt):
                expert_post(e, nt, expert_mm(e, nt))

            # ---- startup interleave: gating L1 (first token-half) and expert
            # 0's first three tile-groups consume each (gx,w0) chunk-pair the
            # moment it lands, keeping the PE fed while DMA is the
            # bottleneck.  PSUM: gating 2x[P,512] + 3 wide expert tiles ----
            hT_sb = [
                constp.tile([P, NLOC], BF16, tag=f"hT{h2}", name=f"hT{h2}")
                for h2 in range(H2)
            ]
            psum_g0 = [
                gatep.tile([P, 512], F32, tag="g", name=f"pg{h2}a")
                for h2 in range(H2)
            ]
            early_psums = [
                widep.tile([P, O], F32, tag="mm", name=f"ps_e0_{nt}")
                for nt in range(3)
            ]
            def e0_mm(dk):
                for nt in range(3):
                    for oh in range(2):
                        nc.tensor.matmul(
                            early_psums[nt][:, oh * 512 : (oh + 1) * 512],
                            xT_sb[dk][:, nt * P : (nt + 1) * P],
                            w_sb[0][:, dk, oh * 512 : (oh + 1) * 512],
                            start=(dk == 0),
                            stop=(dk == DK - 1),
                        )

            for dk in range(DK):
                for h2 in range(H2):
                    nc.tensor.matmul(
                        psum_g0[h2],
                        wg1t_sb[dk][:, h2 * P : (h2 + 1) * P],
                        xT_sb[dk][:, 0:512],
                        start=(dk == 0),
                        stop=(dk == DK - 1),
                    )
                # expert-0's last two chunks are deferred past the second
                # gating pass: gating pass 1 completes as soon as gx lands,
                # and pass 2 (no new data) fills the w0 tail-arrival window
                if dk < DK - 2:
                    e0_mm(dk)
            for h2 in range(H2):
                nc.scalar.activation(
                    out=hT_sb[h2][:, 0:512],
                    in_=psum_g0[h2],
                    func=mybir.ActivationFunctionType.Relu,
                    bias=bg1_sb[:, h2 : h2 + 1],
                )
            # second token-half gating pass (xT fully resident by now)
            psum_g1 = [
                gatep.tile([P, 512], F32, tag="g", name=f"pg{h2}b")
                for h2 in range(H2)
            ]
            for dk in range(DK):
                for h2 in range(H2):
                    nc.tensor.matmul(
                        psum_g1[h2],
                        wg1t_sb[dk][:, h2 * P : (h2 + 1) * P],
                        xT_sb[dk][:, 512:1024],
                        start=(dk == 0),
                        stop=(dk == DK - 1),
                    )
            e0_mm(DK - 2)
            e0_mm(DK - 1)
            for h2 in range(H2):
                nc.scalar.activation(
                    out=hT_sb[h2][:, 512:1024],
                    in_=psum_g1[h2],
                    func=mybir.ActivationFunctionType.Relu,
                    bias=bg1_sb[:, h2 : h2 + 1],
                )

            # ---- gating: logits -> softmax -> gates, vectorized over all
            # tiles in one [P, NT*E] psum.  Logits here are bounded (~±2),
            # so exp needs no max-subtraction in fp32; a single exp + one
            # grouped reduce replaces 8 serialized per-tile chains ----
            for nt in range(NT):
                psum_l = gatep.tile([P, E], F32, tag="g", name=f"psl{nt}")
                for h2 in range(H2):
                    nc.tensor.matmul(
                        psum_l,
                        hT_sb[h2][:, nt * P : (nt + 1) * P],
                        wg2t_sb[:, h2, :],
                        start=(h2 == 0),
                        stop=False,
                    )
                nc.tensor.matmul(psum_l, ones_sb, bg2_sb, start=False, stop=True)

                negmax = workp.tile([P, 1], F32, tag="negmax")
                nc.vector.reduce_max(
                    negmax, psum_l, axis=mybir.AxisListType.X, negate=True
                )
                gates = constp.tile([P, E], F32, tag=f"gates{nt}", name=f"gates{nt}")
                sumexp = workp.tile([P, 1], F32, tag="sumexp")
                nc.scalar.activation(
                    out=gates,
                    in_=psum_l,
                    func=mybir.ActivationFunctionType.Exp,
                    bias=negmax,
                    accum_out=sumexp,
                )
                rsum = workp.tile([P, 1], F32, tag="rsum")
                nc.vector.reciprocal(rsum, sumexp)
                nc.vector.tensor_scalar_mul(gates, gates, rsum)
                gates_sb.append(gates)

            # late, non-critical constants on the GpSimd software DGE
            be_sb = constp.tile([E, O], BF16, tag="be")
            nc.gpsimd.dma_start(out=be_sb, in_=be[:, :])
            ident_sb = constp.tile([P, P], F32, tag="ident")
            nc.gpsimd.dma_start(out=ident_sb, in_=ident[:, :])

            # ---- expert-0 gate weighting + remaining tile-groups ----
            for nt in range(3):
                expert_post(0, nt, early_psums[nt])
            for nt in range(3, NT):
                expert_group(0, nt)

            # ---- main loop: stream experts, accumulate gate-weighted GEMM ----
            for e in range(1, E):
                if e + 1 < E:
                    wtn_r = wt[e + 1].rearrange("(dk p) o -> p dk o", p=P)
                    wn = wpool.tile([P, DK, O], BF16, tag="wh", name=f"w{e+1}_sb")
                    # split across two queues: one queue sustains only
                    # ~115 GB/s, which cuts arrival margins too close
                    nc.sync.dma_start(out=wn[:, 0:4, :], in_=wtn_r[:, 0:4, :])
                    nc.sync.dma_start(out=wn[:, 4:8, :], in_=wtn_r[:, 4:8, :])
                    w_sb[e + 1] = wn
                for nt in range(NT):
                    if e == E - 1 and nt == NT - 1:
                        last_tile_group()
                        continue
                    expert_group(e, nt)
                    if e == 1:
                        # gates.T via PE transpose, interleaved with expert
                        # 1's stream (softmax chain is long done: no stalls)
                        psum_t = gatep.tile([E, P], F32, tag="g", name=f"pst{nt}")
                        nc.tensor.transpose(psum_t, gates_sb[nt], ident_sb)
                        gatesT = constp.tile(
                            [E, P], BF16, tag=f"gatesT{nt}", name=f"gatesT{nt}"
                        )
                        nc.scalar.copy(out=gatesT, in_=psum_t)
                        gatesT_sb.append(gatesT)
                    if e == 2:
                        # bias matmul: psum_b = gates.T @ b_e, added into acc;
                        # half-width psums from the gating pool so the wide
                        # expert-psum rotation keeps its full depth
                        for oh in range(2):
                            sl = slice(oh * 512, (oh + 1) * 512)
                            psum_b = gatep.tile(
                                [P, 512], F32, tag="g", name="psum_b"
                            )
                            nc.tensor.matmul(
                                psum_b, gatesT_sb[nt], be_sb[:, sl],
                                start=True, stop=True,
                            )
                            nc.vector.tensor_add(
                                acc_sb[nt][:, sl], acc_sb[nt][:, sl], psum_b
                            )

    legalize_single_wait(nc)
    return nc


_NC_CACHE = {}


def _get_nc():
    if "nc" not in _NC_CACHE:
        _NC_CACHE["nc"] = build_moe()
    return _NC_CACHE["nc"]


def make_in_maps(x, W_e, b_e, W_g1, b_g1, W_g2, b_g2):
    x = np.asarray(x, dtype=np.float32)
    wt = np.ascontiguousarray(
        np.asarray(W_e, dtype=np.float32).transpose(0, 2, 1)
    ).astype(BF)
    wg1t = np.ascontiguousarray(np.asarray(W_g1, dtype=np.float32).T).astype(BF)
    wg2t = np.ascontiguousarray(np.asarray(W_g2, dtype=np.float32).T).astype(BF)
    bg1 = np.asarray(b_g1, dtype=np.float32)
    bg2 = np.asarray(b_g2, dtype=np.float32).astype(BF)
    be = np.asarray(b_e, dtype=np.float32).astype(BF)
    xb = x.astype(BF)
    ident_np = np.eye(P, dtype=np.float32)
    in_maps = []
    for c in range(NCORES):
        xT_c = np.ascontiguousarray(xb[c * NLOC : (c + 1) * NLOC, :].T)
        gx_c = np.ascontiguousarray(np.concatenate([wg1t, xT_c], axis=1))
        in_maps.append(
            {
                "gx": gx_c,
                "wt": wt,
                "wg2t": wg2t,
                "bg1": bg1,
                "bg2": bg2,
                "be": be,
                "ident": ident_np,
            }
        )
    return in_maps


def kernel(x, W_e, b_e, W_g1, b_g1, W_g2, b_g2, **run_kwargs):
    nc = _get_nc()
    in_maps = make_in_maps(x, W_e, b_e, W_g1, b_g1, W_g2, b_g2)
    res = run_bass_kernel_spmd(nc, in_maps, core_ids=list(range(NCORES)), **run_kwargs)
    out = np.concatenate(
        [res.results[c]["out"].astype(np.float32) for c in range(NCORES)], axis=0
    )
    if run_kwargs:
        kernel.last_results = res
    return out


if __name__ == "__main__":
    rng = np.random.default_rng(0)
    s = 1.0 / np.sqrt(D)
    sh = 1.0 / np.sqrt(H)
    inputs = {
        "x": rng.standard_normal((N, D), dtype=np.float32),
        "W_e": rng.uniform(-s, s, (E, O, D)).astype(np.float32),
        "b_e": rng.uniform(-s, s, (E, O)).astype(np.float32),
        "W_g1": rng.uniform(-s, s, (H, D)).astype(np.float32),
        "b_g1": rng.uniform(-s, s, (H,)).astype(np.float32),
        "W_g2": rng.uniform(-sh, sh, (E, H)).astype(np.float32),
        "b_g2": rng.uniform(-sh, sh, (E,)).astype(np.float32),
    }
    out = kernel(**inputs)
    print("out", out.shape, out.dtype, float(np.abs(out).max()))
